# revision 1
# baseline (speedup 1.0000x reference)
"""Trainium2 Bass kernel for nn_CustomNLLLoss (binary-class NLL with per-class means).

Math: for C=2, the log_softmax picked value obeys
    -picked_i = softplus(x1-x0) if t=0 else softplus(x0-x1)
With d = x1 - x0, g = softplus(d) and softplus(-d) = g - d:
    sum0 = sum_{t=0} g        = S_g - S_tg
    sum1 = sum_{t=1} (g - d)  = S_tg - S_td
    loss = sum0/n0 + sum1/n1
So each core only needs S_g, S_tg, S_td, n1 over its shard, combined on host.

Per-core device work (M = 1M samples as [128 partitions x 8192]):
    POOL: d = x1 - x0 (strided sub; 2x slower than DVE but otherwise idle)
    DVE : 2 fused scalar_tensor_tensor passes (t*g, t*d) whose accum_out
          gives S_tg / S_td per partition; sub for the small tail chunks
    ACT : exp(d), ln(e+1) with accum_out => S_g; copy(t) accum => n1
    DMA : x f32 (8MiB) + targets bf16 (2MiB) in graduated chunks
All per-partition partials stream out as [P, 4, NT]; host does the final fold.
"""

import sys

for _p in ("/opt/trn_rl_repo", "/root/.axon_site/_ro/trn_rl_repo"):
    if _p not in sys.path:
        sys.path.append(_p)

import ml_dtypes
import numpy as np

import concourse.bass as bass
import concourse.tile as tile
from concourse import mybir
from concourse.bass_utils import run_bass_kernel_spmd

N_CORES = 8
N = 8388608
M = N // N_CORES      # samples per core
P = 128               # SBUF partitions
Q = M // P            # per-partition samples per core (8192)

f32 = mybir.dt.float32
bf16 = mybir.dt.bfloat16

# Graduated per-partition chunk sizes: small first chunk so compute
# starts early, shrinking tail so the serial sub->exp->ln->stt chain after
# the final DMA is short.
SIZES = [256, 1024, 1024, 1024, 1024, 1024, 1024, 640, 512, 320, 192, 128]
assert sum(SIZES) == Q
CHUNKS = []
_o = 0
for _s in SIZES:
    CHUNKS.append((_o, _s))
    _o += _s
NT = len(CHUNKS)
DVE_SUB_TAIL = 3      # this many final chunks do their sub on DVE (shorter chain)


def _legalize_waits(nc, max_waits=1):
    """This walrus build rejects instructions carrying more than ~1 sync
    wait ("Too many sync wait commands"), but Tile's Rust wait-assigner
    happily attaches several. Hoist excess waits onto same-engine NOPs
    inserted immediately before the instruction — sequencers execute waits
    in program order, so semantics are unchanged."""
    n = 0
    for f in nc.m.functions:
        for blk in f.blocks:
            il = blk.instructions
            i = 0
            while i < len(il):
                inst = il[i]
                si = getattr(inst, "sync_info", None)
                if si is not None and len(si.on_wait) > max_waits:
                    waits = list(si.on_wait)
                    extra, keep = waits[:-max_waits], waits[-max_waits:]
                    nops = []
                    for w in extra:
                        n += 1
                        nops.append(mybir.InstNoOp(
                            name=f"I-waitfix-{n}",
                            sync_info=mybir.SyncInfo(on_wait=[w], on_update=[]),
                            bass_nofuse=True,
                            engine=inst.engine,
                        ))
                    inst.sync_info = mybir.SyncInfo(
                        on_wait=keep, on_update=list(si.on_update)
                    )
                    il[i:i] = nops
                    i += len(nops)
                i += 1
    return nc


def build_nc():
    nc = bass.Bass("TRN2")
    xs = nc.declare_dram_parameter("xs", [P, Q, 2], f32, isOutput=False)
    ts = nc.declare_dram_parameter("ts", [P, Q], bf16, isOutput=False)
    # stats[:, q, i]: per-partition partial of chunk i
    # (q: 0=S_g, 1=S_tg, 2=S_td); host folds partitions and chunks.
    out = nc.declare_dram_parameter("out", [P, 3, NT], f32, isOutput=True)
    out_t = nc.declare_dram_parameter("out_t", [1, 256], f32, isOutput=True)

    with tile.TileContext(nc) as tc:
        with (
            tc.tile_pool(name="io", bufs=NT) as iop,
            tc.tile_pool(name="wk", bufs=3) as wp,
            tc.tile_pool(name="st", bufs=1) as sp,
            tc.tile_pool(name="ps", bufs=1, space="PSUM") as pp,
        ):
            stats = sp.tile([P, 3, NT], f32)
            ones = sp.tile([P, 1], bf16)
            nc.vector.memset(ones, 1.0)
            psum_t = pp.tile([1, 256], f32)
            nc.vector.memset(psum_t, 0.0)

            for i, (o0, sz) in enumerate(CHUNKS):
                xt = iop.tile([P, sz, 2], f32, tag="x")
                tt = iop.tile([P, sz], bf16, tag="t")
                nc.sync.dma_start(out=xt, in_=xs[:, o0 : o0 + sz, :])
                nc.sync.dma_start(out=tt, in_=ts[:, o0 : o0 + sz])

                # sub on POOL (otherwise idle) keeps DVE under the DMA
                # roofline; tail chunks sub on DVE for a shorter chain.
                d = wp.tile([P, sz], f32, tag="d")
                sub_eng = nc.vector if i >= NT - DVE_SUB_TAIL else nc.gpsimd
                sub_eng.tensor_tensor(
                    out=d, in0=xt[:, :, 1], in1=xt[:, :, 0],
                    op=mybir.AluOpType.subtract,
                )
                s2 = wp.tile([P, sz], f32, tag="s2")
                nc.vector.scalar_tensor_tensor(
                    out=s2, in0=tt, scalar=1.0, in1=d,
                    op0=mybir.AluOpType.mult, op1=mybir.AluOpType.mult,
                    accum_out=stats[:, 2, i : i + 1],
                )
                e = wp.tile([P, sz], f32, tag="e")
                nc.scalar.activation(
                    out=e, in_=d, func=mybir.ActivationFunctionType.Exp,
                )
                g = wp.tile([P, sz], f32, tag="g")
                nc.scalar.activation(
                    out=g, in_=e,
                    func=mybir.ActivationFunctionType.Ln,
                    bias=1.0, scale=1.0,
                    accum_out=stats[:, 0, i : i + 1],
                )
                s1 = wp.tile([P, sz], f32, tag="s1")
                nc.vector.scalar_tensor_tensor(
                    out=s1, in0=tt, scalar=1.0, in1=g,
                    op0=mybir.AluOpType.mult, op1=mybir.AluOpType.mult,
                    accum_out=stats[:, 1, i : i + 1],
                )
                # n1 partials on the otherwise-idle PE: ones^T @ t subchunks
                # accumulated into one pre-zeroed PSUM bank (start=False).
                for c0 in range(0, sz, 256):
                    cn = min(256, sz - c0)
                    nc.tensor.matmul(
                        psum_t[:, 0:cn],
                        lhsT=ones,
                        rhs=tt[:, c0 : c0 + cn],
                        start=False,
                        stop=(i == NT - 1 and c0 + 256 >= sz),
                        skip_group_check=True,
                    )

            tshow = sp.tile([1, 256], f32)
            nc.scalar.copy(out=tshow, in_=psum_t)
            nc.sync.dma_start(out=out_t[:, :], in_=tshow)
            nc.sync.dma_start(out=out[:, :, :], in_=stats)
    return _legalize_waits(nc)


_NC = None


def get_nc():
    global _NC
    if _NC is None:
        _NC = build_nc()
    return _NC


def run_device(x, tb, **spmd_kwargs):
    """x: [N,2] f32 contiguous, tb: [N] bfloat16. Returns (sums[4] float64, results)."""
    in_maps = []
    for c in range(N_CORES):
        in_maps.append({
            "xs": x[c * M : (c + 1) * M].reshape(P, Q, 2),
            "ts": tb[c * M : (c + 1) * M].reshape(P, Q),
        })
    res = run_bass_kernel_spmd(get_nc(), in_maps, list(range(N_CORES)), **spmd_kwargs)
    stats = np.stack([r["out"] for r in res.results]).astype(np.float64)
    tsum = np.stack([r["out_t"] for r in res.results]).astype(np.float64)
    s_g, s_tg, s_td = stats.sum(axis=(0, 1, 3))
    n1 = tsum.sum()
    return np.array([s_g, s_tg, s_td, n1]), res


def kernel(x, targets):
    x = np.ascontiguousarray(np.asarray(x), dtype=np.float32)
    tb = np.asarray(targets).astype(ml_dtypes.bfloat16)  # 0/1 exact in bf16
    (s_g, s_tg, s_td, n1), _ = run_device(x, tb)
    sum0 = s_g - s_tg
    sum1 = s_tg - s_td
    n0 = float(N) - n1
    p = sum0 / n0 if n0 > 0 else 0.0
    r = sum1 / n1 if n1 > 0 else 0.0
    return np.array(p + r, dtype=np.float32)

